# revision 12
# baseline (speedup 1.0000x reference)
"""3-layer GAT (PyG GATConv semantics) on 8 trn2 NeuronCores via Bass/Tile.

Distribution: nodes dst-sharded across the 8 cores (12500 nodes each).
Per layer: local node-phase matmul h_aug = x @ [W | W@As | W@Ad] (fp32),
stored bf16 into a shard with 256-elem rows ([h(128) | ls(4) | ld(4) | pad]),
AllGather into a full DRAM table, then an edge phase over windows of 128
destination nodes. Edge payloads are fetched with batched SWDGE dma_gather
instructions (one per window x core-pair region, int16 indices, spread over
4 SWDGE queues), the attention logit's dst half (alpha_d) is broadcast to
edge slots on-chip via PE-transposed one-hot matrices against alpha_d
stashed during the node phase, and the segment softmax + scatter runs as
bf16 PE matmuls ps[:,:fh] += Q^T @ (p*h), ps[:,fh:] += Q^T @ p accumulated
in PSUM per window.
"""
import sys
if '/opt/trn_rl_repo' not in sys.path:
    sys.path.insert(0, '/opt/trn_rl_repo')
import numpy as np
from concourse import bass, mybir, tile, bacc

F32 = mybir.dt.float32
BF16 = mybir.dt.bfloat16
I16 = mybir.dt.int16
P = 128
NCORES = 8
N_NODES = 100000
NSWQ = 4                   # SWDGE queues for dma_gather round-robin
LAST_EXEC_NS = None


# ------------------------------------------------------------- profiling shim

def _install_ntff_hook():
    """Best-effort: register the axon NTFF profile hook if the image's antenv
    lacks it. Returns True if trace=True is usable."""
    try:
        from antenv.axon_hooks import get_axon_ntff_profile_hook  # noqa: F401
        return True
    except ImportError:
        pass
    try:
        import types, contextlib, ctypes, os, json, uuid
        path = "/root/.axon_site/trn_agent_boot/trn_boot.py"
        so = "/opt/axon/libaxon_pjrt.so"
        if not (os.path.exists(path) and os.path.exists(so)):
            return False
        srclines = open(path).read().splitlines()
        start = next(i for i, l in enumerate(srclines)
                     if l.startswith("def _ntff_profile_via_ctypes"))
        end = start + 1
        while end < len(srclines) and (srclines[end].startswith((" ", "\t"))
                                       or not srclines[end].strip()):
            end += 1
        ns = dict(contextlib=contextlib, ctypes=ctypes, sys=sys, os=os,
                  json=json, uuid=uuid)
        exec("\n".join(srclines[start:end]), ns)
        hook = ns["_ntff_profile_via_ctypes"](so)
        mod = types.ModuleType("antenv.axon_hooks")
        mod.get_axon_ntff_profile_hook = lambda: hook
        mod.set_axon_ntff_profile_hook = lambda h: None
        sys.modules["antenv.axon_hooks"] = mod
        return hook is not None
    except Exception:
        return False


# ---------------------------------------------------------------- host side

def preprocess(src, dst, N, ncores):
    """Slot layout: edges grouped by (window w, core-pair q); each (w,q)
    segment padded to a multiple of 128 slots (uniform across cores) so a
    window's pay columns are contiguous: [w: q0|q1|q2|q3][w+1: ...].
    Segment slot i -> (partition i%128, column segbase + i//128).
    One dma_gather call per (w, q)."""
    nloc = N // ncores
    assert nloc * ncores == N
    nwin = (nloc + P - 1) // P
    npad = nwin * P
    npad_s = npad + 16
    dummy_loc = npad                   # dummy row offset inside each region
    trows = ncores * npad_s
    npair = ncores // 2
    pairrows = 2 * npad_s

    src = src.astype(np.int64)
    dst = dst.astype(np.int64)
    core = dst // nloc
    dloc = dst - core * nloc
    win = dloc // P
    rel = (dloc - win * P).astype(np.float32)
    grow = (npad_s * (src // nloc) + (src % nloc)).astype(np.int64)
    pair = grow // pairrows
    lrow = (grow - pair * pairrows).astype(np.int64)

    gid = (core * nwin + win) * npair + pair
    counts = np.bincount(gid, minlength=ncores * nwin * npair)
    counts = counts.reshape(ncores, nwin, npair)
    cols = (counts.max(axis=0) + P - 1) // P          # [nwin, npair]
    assert cols.max() * P <= 1024, "per-call descriptor limit"

    colbase = np.zeros((nwin, npair), np.int64)
    cb = 0
    for w in range(nwin):
        for q in range(npair):
            colbase[w, q] = cb
            cb += int(cols[w, q])
    totcol = cb
    cidx = totcol * 8

    i16 = np.full((ncores, 16, cidx), dummy_loc, np.int16)
    rel_f = np.full((ncores, P, totcol), -1.0, np.float32)

    order = np.argsort(gid, kind='stable')
    sorted_gid = gid[order]
    grp_start = np.concatenate([[0], np.flatnonzero(np.diff(sorted_gid)) + 1])
    grp_of = np.repeat(np.arange(len(grp_start)),
                       np.diff(np.concatenate([grp_start, [len(order)]])))
    rank = np.empty(len(order), np.int64)
    rank[order] = np.arange(len(order)) - grp_start[grp_of]

    part = rank % P
    col = colbase[win, pair] + rank // P
    rel_f[core, part, col] = rel
    callpos = (col - colbase[win, pair]) * P + part
    idxcol = colbase[win, pair] * 8 + callpos // 16
    i16[core, callpos % 16, idxcol] = lrow

    wincalls = []
    jw = []
    wcol0 = []
    for w in range(nwin):
        cl = []
        lc = 0
        for q in range(npair):
            c = int(cols[w, q])
            cl.append((int(colbase[w, q] * 8), c * P, lc))
            lc += c
        wincalls.append(cl)
        jw.append(lc)
        wcol0.append(int(colbase[w, 0]))

    meta = dict(N=N, ncores=ncores, nloc=nloc, nwin=int(nwin), npad=int(npad),
                npad_s=int(npad_s), trows=int(trows),
                npair=int(npair), pairrows=int(pairrows),
                totcol=int(totcol), cidx=int(cidx),
                wincalls=wincalls, jw=jw, wcol0=wcol0)
    return meta, i16, rel_f


def make_weights(inp):
    def aug(W, a_s, a_d):
        H, C = a_s.shape
        As = np.zeros((H * C, H), np.float32)
        Ad = np.zeros((H * C, H), np.float32)
        for h in range(H):
            As[h * C:(h + 1) * C, h] = a_s[h]
            Ad[h * C:(h + 1) * C, h] = a_d[h]
        return np.concatenate([W, W @ As, W @ Ad], axis=1).astype(np.float32)
    return (aug(np.asarray(inp['W0'], np.float32), np.asarray(inp['as0']),
                np.asarray(inp['ad0'])),
            aug(np.asarray(inp['W1'], np.float32), np.asarray(inp['as1']),
                np.asarray(inp['ad1'])),
            aug(np.asarray(inp['W2'], np.float32), np.asarray(inp['as2']),
                np.asarray(inp['ad2'])))


def make_const_inputs(inp):
    from ml_dtypes import bfloat16
    Waug0, Waug1, Waug2 = make_weights(inp)
    FA, FA2 = 136, 34
    dummy = np.zeros((16, FA), np.float32)
    dummy[:, 128:132] = -1e30
    dummy2 = np.zeros((16, FA2), np.float32)
    dummy2[:, 32:33] = -1e30
    tl = lambda a: np.tile(np.asarray(a, np.float32).reshape(1, -1), (P, 1))
    return dict(
        Waug0=Waug0, Waug1=Waug1, Waug2=Waug2,
        b0=tl(inp['b0']), b1=tl(inp['b1']), b2=tl(inp['b2']),
        linw=np.asarray(inp['lin_w'], np.float32),
        linb=tl(inp['lin_b']),
        iota=np.tile(np.arange(P, dtype=np.float32).reshape(1, P),
                     (P, 1)).astype(bfloat16),
        dummyrow=dummy.astype(bfloat16), dummyrow2=dummy2.astype(bfloat16),
        ident=np.eye(P, dtype=np.float32),
        ident16=np.eye(P, dtype=np.float32).astype(bfloat16),
    )


# ---------------------------------------------------------------- device side

def ap_nd(t_ap, off, dims):
    """AP over the same tensor: keep partition dim, explicit free dims."""
    ap = [list(t_ap.ap[0])] + [[int(s), int(n)] for (s, n) in dims]
    return bass.AP(t_ap.tensor, t_ap.offset + off, ap)


def build_program(meta, ncores=None):
    ncores = ncores or meta['ncores']
    nwin, npad, npad_s, trows = (meta['nwin'], meta['npad'], meta['npad_s'],
                                 meta['trows'])
    npair, pairrows = meta['npair'], meta['pairrows']
    totcol, cidx = meta['totcol'], meta['cidx']
    wincalls, JW, wcol0 = meta['wincalls'], meta['jw'], meta['wcol0']
    FH, H, C = 128, 4, 32
    FA = FH + 2 * H            # 136 written cols, layers 0/1
    FA2 = C + 2                # 34 written cols, layer 2
    E01, E2 = 256, 128         # table row elems (bf16)
    NCLS = 40

    nc = bacc.Bacc("TRN2", target_bir_lowering=False, debug=False,
                   num_devices=ncores, num_swdge_queues=NSWQ)
    dp = nc.declare_dram_parameter
    xT = dp("xT", [P, npad], F32, isOutput=False)
    idx_d = dp("idx16", [P, cidx], I16, isOutput=False)
    rel_d = dp("rel", [P, totcol], BF16, isOutput=False)
    Waug0_d = dp("Waug0", [P, FA], F32, isOutput=False)
    Waug1_d = dp("Waug1", [P, FA], F32, isOutput=False)
    Waug2_d = dp("Waug2", [P, FA2], F32, isOutput=False)
    b0_d = dp("b0", [P, FH], F32, isOutput=False)
    b1_d = dp("b1", [P, FH], F32, isOutput=False)
    b2_d = dp("b2", [P, C], F32, isOutput=False)
    linw_d = dp("linw", [C, NCLS], F32, isOutput=False)
    linb_d = dp("linb", [P, NCLS], F32, isOutput=False)
    iota_d = dp("iota", [P, P], BF16, isOutput=False)
    dummy_d = dp("dummyrow", [16, FA], BF16, isOutput=False)
    dummy2_d = dp("dummyrow2", [16, FA2], BF16, isOutput=False)
    ident_d = dp("ident", [P, P], F32, isOutput=False)
    ident16_d = dp("ident16", [P, P], BF16, isOutput=False)
    out_ext = dp("out", [npad, NCLS], F32, isOutput=True)

    rg = [list(range(ncores))]
    qctr = [0]

    with tile.TileContext(nc) as tc:
        with tc.tile_pool(name="dram", bufs=1, space="DRAM") as dram, \
             tc.tile_pool(name="consts", bufs=1) as cp, \
             tc.tile_pool(name="work", bufs=3) as wp, \
             tc.tile_pool(name="pay", bufs=4) as wpay, \
             tc.tile_pool(name="psum", bufs=2, space="PSUM") as pp:

            table0 = dram.tile([trows, E01], BF16, addr_space="Shared",
                               name="table0")
            table1 = dram.tile([trows, E01], BF16, addr_space="Shared",
                               name="table1")
            table2 = dram.tile([trows, E2], BF16, addr_space="Shared",
                               name="table2")
            shard0 = dram.tile([npad_s, E01], BF16, name="shard0")
            shard1 = dram.tile([npad_s, E01], BF16, name="shard1")
            shard2 = dram.tile([npad_s, E2], BF16, name="shard2")

            def cload(dram_ap, shape, name, dtype=F32):
                t = cp.tile(shape, dtype, name=name, tag=name)
                nc.sync.dma_start(out=t[:], in_=dram_ap)
                return t
            Waug0_s = cload(Waug0_d[:], [P, FA], "Waug0_s")
            Waug1_s = cload(Waug1_d[:], [P, FA], "Waug1_s")
            Waug2_s = cload(Waug2_d[:], [P, FA2], "Waug2_s")
            b0_s = cload(b0_d[:], [P, FH], "b0_s")
            b1_s = cload(b1_d[:], [P, FH], "b1_s")
            b2_s = cload(b2_d[:], [P, C], "b2_s")
            linw_s = cload(linw_d[:], [C, NCLS], "linw_s")
            linb_s = cload(linb_d[:], [P, NCLS], "linb_s")
            iota_s = cload(iota_d[:], [P, P], "iota_s", BF16)
            dummy_s = cload(dummy_d[:], [16, FA], "dummy_s", BF16)
            dummy2_s = cload(dummy2_d[:], [16, FA2], "dummy2_s", BF16)
            ident_s = cload(ident_d[:], [P, P], "ident_s")
            ident16_s = cload(ident16_d[:], [P, P], "ident16_s", BF16)
            idx_s = cload(idx_d[:], [P, cidx], "idx_s", I16)
            rel_s = cload(rel_d[:], [P, totcol], "rel_s", BF16)

            adst0 = cp.tile([P, nwin * H], BF16, tag="adst0")
            adst1 = cp.tile([P, nwin * H], BF16, tag="adst1")
            adst2 = cp.tile([P, nwin * 1], BF16, tag="adst2")

            # node phase layer 0
            for blk in range(nwin):
                xT_t = wp.tile([P, P], F32, tag="xT_t")
                nc.sync.dma_start(out=xT_t[:], in_=xT[:, blk * P:(blk + 1) * P])
                ps = pp.tile([P, FA], F32, tag="tail")
                nc.tensor.matmul(out=ps[:], lhsT=xT_t[:], rhs=Waug0_s[:],
                                 start=True, stop=True)
                hsb = wp.tile([P, FA], BF16, tag="hsb")
                nc.scalar.copy(out=hsb[:], in_=ps[:])
                nc.scalar.copy(out=adst0[:, blk * H:(blk + 1) * H],
                               in_=ps[:, FH + H:FH + 2 * H])
                nc.sync.dma_start(out=shard0[blk * P:(blk + 1) * P, :FA],
                                  in_=hsb[:])
            nc.sync.dma_start(out=shard0[npad:npad + 16, :FA], in_=dummy_s[:])

            def allgather(shard, table):
                nc.gpsimd.collective_compute(
                    "AllGather", mybir.AluOpType.bypass,
                    replica_groups=rg, ins=[shard.opt()], outs=[table.opt()])

            allgather(shard0, table0)

            def edge_phase(table, E, nheads, adst, shard_next, FN, Waug_next_s,
                           b_s, adst_next, nh_next, final):
                ch = C
                fh = nheads * ch
                cols = fh + nheads
                ls_off = fh
                for w in range(nwin):
                    Jw = JW[w]
                    gc0 = wcol0[w]
                    pay = wpay.tile([P, Jw * E], BF16, tag="pay")
                    for q in range(npair):
                        ic0, n, lc0 = wincalls[w][q]
                        nc.gpsimd.dma_gather(
                            out_ap=pay[:, lc0 * E:(lc0 + n // P) * E]
                            .rearrange("p (s e) -> p s e", e=E),
                            in_ap=table[q * pairrows:(q + 1) * pairrows, :],
                            idxs_ap=idx_s[:, ic0:ic0 + n // 16],
                            num_idxs=n, num_idxs_reg=n, elem_size=E,
                            queue_num=qctr[0] % NSWQ)
                        qctr[0] += 1
                    Q = wp.tile([P, Jw * P], BF16, tag="Q")
                    nc.vector.tensor_tensor(
                        out=Q[:].rearrange("p (j w) -> p j w", j=Jw),
                        in0=ap_nd(rel_s[:], gc0, [(1, Jw), (0, P)]),
                        in1=ap_nd(iota_s[:], 0, [(0, Jw), (1, P)]),
                        op=mybir.AluOpType.is_equal)
                    QT = wp.tile([P, Jw * P], BF16, tag="QT")
                    for j0 in range(0, Jw, 8):
                        jn = min(8, Jw - j0)
                        qt_ps = pp.tile([P, jn * P], BF16, tag="qt_ps")
                        for j in range(jn):
                            nc.tensor.transpose(
                                out=qt_ps[:, j * P:(j + 1) * P],
                                in_=Q[:, (j0 + j) * P:(j0 + j + 1) * P],
                                identity=ident16_s[:])
                        nc.scalar.copy(
                            out=QT[:, j0 * P:(j0 + jn) * P], in_=qt_ps[:])
                    edv = pp.tile([P, Jw * nheads], F32, tag="edv")
                    for j in range(Jw):
                        nc.tensor.matmul(
                            out=edv[:, j * nheads:(j + 1) * nheads],
                            lhsT=QT[:, j * P:(j + 1) * P],
                            rhs=adst[:, w * nheads:(w + 1) * nheads],
                            start=True, stop=True)
                    lg = wp.tile([P, Jw * nheads], F32, tag="lg")
                    nc.vector.tensor_tensor(
                        out=lg[:],
                        in0=ap_nd(pay[:], ls_off, [(E, Jw), (1, nheads)]),
                        in1=edv[:], op=mybir.AluOpType.add)
                    nc.vector.scalar_tensor_tensor(
                        out=lg[:], in0=lg[:], scalar=0.2, in1=lg[:],
                        op0=mybir.AluOpType.mult, op1=mybir.AluOpType.max)
                    pv = wp.tile([P, Jw * nheads], BF16, tag="pv")
                    nc.scalar.activation(out=pv[:], in_=lg[:],
                                         func=mybir.ActivationFunctionType.Exp)
                    rh = wp.tile([P, Jw * fh], BF16, tag="rh")
                    for hh in range(nheads):
                        nc.vector.tensor_tensor(
                            out=ap_nd(rh[:], hh * ch, [(fh, Jw), (1, ch)]),
                            in0=ap_nd(pay[:], hh * ch, [(E, Jw), (1, ch)]),
                            in1=ap_nd(pv[:], hh, [(nheads, Jw), (0, ch)]),
                            op=mybir.AluOpType.mult)
                    ps = pp.tile([P, cols], F32, tag="ps_edge")
                    for j in range(Jw):
                        nc.tensor.matmul(
                            out=ps[:, :fh], lhsT=Q[:, j * P:(j + 1) * P],
                            rhs=rh[:, j * fh:(j + 1) * fh],
                            start=(j == 0), stop=(j == Jw - 1))
                    for j in range(Jw):
                        nc.tensor.matmul(
                            out=ps[:, fh:fh + nheads],
                            lhsT=Q[:, j * P:(j + 1) * P],
                            rhs=pv[:, j * nheads:(j + 1) * nheads],
                            start=(j == 0), stop=(j == Jw - 1))
                    dn = wp.tile([P, nheads], F32, tag="dn")
                    nc.vector.tensor_scalar_add(dn[:], ps[:, fh:fh + nheads],
                                                1e-16)
                    rc = wp.tile([P, nheads], F32, tag="rc")
                    nc.vector.reciprocal(rc[:], dn[:])
                    xr = wp.tile([P, fh], F32, tag="xr")
                    nc.vector.tensor_tensor(
                        out=ap_nd(xr[:], 0, [(ch, nheads), (1, ch)]),
                        in0=ap_nd(ps[:], 0, [(ch, nheads), (1, ch)]),
                        in1=ap_nd(rc[:], 0, [(1, nheads), (0, ch)]),
                        op=mybir.AluOpType.mult)
                    nc.vector.tensor_tensor(
                        out=xr[:], in0=xr[:], in1=b_s[:, :fh],
                        op=mybir.AluOpType.add)
                    nc.vector.tensor_scalar_max(xr[:], xr[:], 0.0)
                    pst = pp.tile([P, 136], F32, tag="tail")
                    nc.tensor.transpose(out=pst[:fh, :P], in_=xr[:],
                                        identity=ident_s[:])
                    xrT = wp.tile([fh, P], F32, tag="xrT")
                    nc.scalar.copy(out=xrT[:], in_=pst[:fh, :P])
                    if not final:
                        psn = pp.tile([P, 136], F32, tag="tail")
                        nc.tensor.matmul(out=psn[:, :FN], lhsT=xrT[:],
                                         rhs=Waug_next_s[:], start=True,
                                         stop=True)
                        hn = wp.tile([P, FN], BF16, tag="hn")
                        nc.scalar.copy(out=hn[:], in_=psn[:, :FN])
                        fhn = FN - 2 * nh_next
                        nc.scalar.copy(
                            out=adst_next[:, w * nh_next:(w + 1) * nh_next],
                            in_=psn[:, fhn + nh_next:fhn + 2 * nh_next])
                        nc.sync.dma_start(
                            out=shard_next[w * P:(w + 1) * P, :FN],
                            in_=hn[:])
                    else:
                        psn = pp.tile([P, 136], F32, tag="tail")
                        nc.tensor.matmul(out=psn[:, :NCLS], lhsT=xrT[:],
                                         rhs=linw_s[:], start=True,
                                         stop=True)
                        yo = wp.tile([P, NCLS], F32, tag="yo")
                        nc.vector.tensor_tensor(
                            out=yo[:], in0=psn[:, :NCLS], in1=linb_s[:],
                            op=mybir.AluOpType.add)
                        nc.sync.dma_start(out=out_ext[w * P:(w + 1) * P, :],
                                          in_=yo[:])

            edge_phase(table0, E01, H, adst0, shard1, FA, Waug1_s, b0_s,
                       adst1, H, final=False)
            nc.sync.dma_start(out=shard1[npad:npad + 16, :FA], in_=dummy_s[:])
            allgather(shard1, table1)
            edge_phase(table1, E01, H, adst1, shard2, FA2, Waug2_s, b1_s,
                       adst2, 1, final=False)
            nc.sync.dma_start(out=shard2[npad:npad + 16, :FA2], in_=dummy2_s[:])
            allgather(shard2, table2)
            edge_phase(table2, E2, 1, adst2, None, None, None, b2_s,
                       None, None, final=True)

    nc.compile()
    return nc


# ---------------------------------------------------------------- entry point

def kernel(**inputs):
    from concourse.bass_utils import run_bass_kernel_spmd
    from ml_dtypes import bfloat16
    global LAST_EXEC_NS
    N = N_NODES
    ncores = NCORES
    x = np.asarray(inputs['x'], np.float32)
    ei = np.asarray(inputs['edge_index'])
    loop = np.arange(N, dtype=np.int64)
    src = np.concatenate([np.asarray(ei[0], np.int64), loop])
    dst = np.concatenate([np.asarray(ei[1], np.int64), loop])
    meta, i16, rel_f = preprocess(src, dst, N, ncores)
    consts = make_const_inputs(inputs)
    nloc, npad = meta['nloc'], meta['npad']

    nc = build_program(meta, ncores)

    in_maps = []
    for c in range(ncores):
        xc = np.zeros((npad, 128), np.float32)
        xc[:nloc] = x[c * nloc:(c + 1) * nloc]
        m = dict(consts)
        m['xT'] = np.ascontiguousarray(xc.T)
        m['idx16'] = np.ascontiguousarray(np.tile(i16[c], (8, 1)))
        m['rel'] = np.ascontiguousarray(rel_f[c]).astype(bfloat16)
        in_maps.append(m)

    trace = _install_ntff_hook()
    res = run_bass_kernel_spmd(nc, in_maps, list(range(ncores)), trace=trace)
    LAST_EXEC_NS = res.exec_time_ns
    out = np.concatenate(
        [res.results[c]['out'][:nloc] for c in range(ncores)], axis=0)
    return np.ascontiguousarray(out.astype(np.float32))
